# revision 7
# baseline (speedup 1.0000x reference)
"""FAGCN (2-layer FAConv GNN) Trainium2 kernel, 8 NeuronCores SPMD.

Sharding: nodes by id-range across 8 cores (12500 each); edges partitioned by
dst so segment-sum is local; per-layer halo exchange = AllGather of the
per-node table; small weights replicated.

Table row (fp16 x 256 = 512B): [dinv*h hi (128) | al hi | al lo | dinv*h lo
(126, features 0..125)] - hi+lo fp16 pairs carry ~22-bit mantissa (~f32).
dinv_src is folded into the table rows and dinv_dst into the PSUM evacuation,
so the per-edge coefficient is just tanh(al_src + ar_dst).

Per-core pipeline:
  A) h = x @ W_in + b_in (f32 PE matmuls from host-transposed x), h0e=eps*h
  B) per layer: al/ar row-dots, build table shard -> AllGather -> full table
  C) per layer, per (supertile x src-bank) call: dma_gather 512B rows by edge
     src (int16 bank-local ids), reconstruct f32 messages, Z = ar_row + al_s,
     tanh on ScalarE, M = onehot(dstloc) * tanh(Z), f32 one-hot matmul
     segment-sum into PSUM, evac dinv_d*agg + eps*h0 (+relu for layer 1)
  D) logits / softmax / argmax head
"""
import sys
import numpy as np

for _p in ('/opt/trn_rl_repo', '/root/.axon_site'):
    if _p not in sys.path:
        sys.path.insert(0, _p)

from concourse import bass, mybir  # noqa: E402
import concourse.tile as tile  # noqa: E402
from concourse import bacc  # noqa: E402
from concourse.masks import make_identity  # noqa: E402
from concourse.bass_utils import run_bass_kernel_spmd  # noqa: E402

F32 = mybir.dt.float32
F16 = mybir.dt.float16
I16 = mybir.dt.int16
U32 = mybir.dt.uint32
AF = mybir.ActivationFunctionType
ALU = mybir.AluOpType

N, E, IN, H, C = 100000, 1600000, 500, 128, 40
EPS = 0.1
NC_ = 8                     # cores
PC = 12500                  # real nodes per core
NT = 98                     # dst tiles per core
PCP = NT * 128              # 12544 padded nodes per core
GN = NC_ * PCP              # 100352 padded global rows
NBANK = 4
BROWS = GN // NBANK         # 25088 rows per bank (< 32768 for int16 idx)
SUP = 4                     # dst tiles per supertile
RW = 256                    # fp16 elems per table row (512B)
INP = 512                   # padded input dim

LAST_EXEC_NS = None


def _enable_trace():
    try:
        import types
        import antenv
        if 'antenv.axon_hooks' not in sys.modules:
            hm = types.ModuleType('antenv.axon_hooks')
            _h = {}
            hm.set_axon_ntff_profile_hook = lambda h: _h.__setitem__('h', h)
            hm.get_axon_ntff_profile_hook = lambda: _h.get('h')
            sys.modules['antenv.axon_hooks'] = hm
            antenv.axon_hooks = hm
            from trn_agent_boot.trn_boot import _ntff_profile_via_ctypes
            hook = _ntff_profile_via_ctypes('/opt/axon/libaxon_pjrt.so')
            if hook is not None:
                hm.set_axon_ntff_profile_hook(hook)
        return sys.modules['antenv.axon_hooks'].get_axon_ntff_profile_hook() is not None
    except Exception:
        return False


def _wrap_idxs(idx):
    n = len(idx)
    S = -(-n // 16)
    flat = np.zeros(S * 16, np.int64)
    flat[:n] = idx
    buf = flat.reshape(S, 16).T.astype(np.int16)
    return np.tile(buf, (8, 1))


def _prep_edges(edge_index):
    src = np.concatenate([edge_index[0], np.arange(N, dtype=np.int64)])
    dst = np.concatenate([edge_index[1], np.arange(N, dtype=np.int64)])
    deg = np.bincount(dst, minlength=N).astype(np.float64)
    dinv = (1.0 / np.sqrt(deg)).astype(np.float32)
    row = (src // PC) * PCP + (src % PC)
    core = dst // PC

    NS = (NT + SUP - 1) // SUP
    counts = np.zeros((NC_, NS, NBANK, SUP), np.int64)
    buckets = [[[[None] * SUP for _ in range(NBANK)] for _ in range(NS)]
               for _ in range(NC_)]
    for k in range(NC_):
        m = core == k
        er, ed = row[m], (dst[m] - k * PC).astype(np.int64)
        t = ed >> 7
        b = er // BROWS
        key = (t // SUP) * (NBANK * SUP) + b * SUP + (t % SUP)
        order = np.argsort(key, kind='stable')
        er, ed, key = er[order], ed[order], key[order]
        bnd = np.searchsorted(key, np.arange(NS * NBANK * SUP + 1))
        for s in range(NS):
            for b_ in range(NBANK):
                for ti in range(SUP):
                    kk = s * (NBANK * SUP) + b_ * SUP + ti
                    lo, hi = bnd[kk], bnd[kk + 1]
                    counts[k, s, b_, ti] = hi - lo
                    buckets[k][s][b_][ti] = (er[lo:hi] - b_ * BROWS,
                                             ed[lo:hi] - (s * SUP + ti) * 128)
    mx = counts.max(axis=0)
    ntile = -(-mx // 128)
    calls = []
    tile_off = 0
    idx_coloff = 0
    for s in range(NS):
        sup_w = min(SUP, NT - s * SUP)
        for b_ in range(NBANK):
            tl = [int(ntile[s, b_, ti]) for ti in range(sup_w)]
            ntl = sum(tl)
            if ntl == 0:
                continue
            calls.append((s, b_, tile_off, tl, idx_coloff, ntl * 128))
            tile_off += ntl
            idx_coloff += ntl * 8
    TT, GW = tile_off, idx_coloff
    per_core = []
    for k in range(NC_):
        gidx = np.zeros((128, GW), np.int16)
        dstloc = np.zeros((128, TT), np.float32)
        for (s, b_, toff, tl, ioff, nidx) in calls:
            ii = np.zeros(nidx, np.int64)
            dd = np.full(nidx, -1.0, np.float64)   # pad edges match no column
            pos = 0
            for ti, ntl_t in enumerate(tl):
                er, ed = buckets[k][s][b_][ti]
                ii[pos:pos + len(er)] = er
                dd[pos:pos + len(ed)] = ed
                pos += ntl_t * 128
            gidx[:, ioff:ioff + nidx // 16] = _wrap_idxs(ii)
            dstloc[:, toff:toff + nidx // 128] = dd.reshape(-1, 128).T
        per_core.append(dict(gidx=gidx, dstloc=dstloc,
                             dinv=np.ascontiguousarray(
                                 np.pad(dinv[k * PC:(k + 1) * PC],
                                        (0, PCP - PC), constant_values=1.0)
                                 .reshape(NT, 128).T)))
    return dict(calls=calls, TT=TT, GW=GW, NS=NS), per_core


def _build_nc(meta):
    nc = bacc.Bacc("TRN2", target_bir_lowering=False, debug=False,
                   num_devices=NC_, num_swdge_queues=4)
    xt = nc.dram_tensor("xt", [4, 128, PCP], F32, kind="ExternalInput")
    w_in = nc.dram_tensor("w_in", [4, 128, H], F32, kind="ExternalInput")
    b_in_d = nc.dram_tensor("b_in", [128, 1], F32, kind="ExternalInput")
    att_d = nc.dram_tensor("att", [4, 128, H], F32, kind="ExternalInput")
    w_cls_d = nc.dram_tensor("w_cls", [128, C], F32, kind="ExternalInput")
    b_cls_d = nc.dram_tensor("b_cls", [128, C], F32, kind="ExternalInput")
    gidx_d = nc.dram_tensor("gidx", [128, meta["GW"]], I16, kind="ExternalInput")
    dstloc_d = nc.dram_tensor("dstloc", [128, meta["TT"]], F32,
                              kind="ExternalInput")
    dinv_d = nc.dram_tensor("dinv", [128, NT], F32, kind="ExternalInput")
    logits_d = nc.dram_tensor("logits", [PCP, C], F32, kind="ExternalOutput")
    emb_d = nc.dram_tensor("emb", [PCP, H], F32, kind="ExternalOutput")
    soft_d = nc.dram_tensor("soft", [PCP, C], F32, kind="ExternalOutput")
    hard_d = nc.dram_tensor("hard", [PCP, 8], U32, kind="ExternalOutput")

    calls = meta["calls"]
    last_mm = {}
    for ci, (s, b_, toff, tl, ioff, nidx) in enumerate(calls):
        for ti, w in enumerate(tl):
            if w > 0:
                last_mm[(s, ti)] = (ci, ti, w - 1)

    with tile.TileContext(nc) as tc:
        with tc.tile_pool(name="persist", bufs=1) as pp, \
             tc.tile_pool(name="dram", bufs=1, space="DRAM") as dp:
            cur_nm = pp.tile([128, NT, H], F32, tag="cur")
            dstloc_sb = pp.tile([128, meta["TT"]], F32, tag="dstloc")
            dinv_sb = pp.tile([128, NT], F32, tag="dinv")
            iota_sb = pp.tile([128, 128], F32, tag="iota")
            ident = pp.tile([128, 128], F32, tag="ident")
            atts = [pp.tile([128, H], F32, tag=f"att{i}", name=f"att{i}")
                    for i in range(4)]
            b_in_sb = pp.tile([128, 1], F32, tag="b_in")
            w_cls_sb = pp.tile([128, C], F32, tag="w_cls")
            b_cls_sb = pp.tile([128, C], F32, tag="b_cls")
            ar_nm = pp.tile([128, NT], F32, tag="ar_nm")
            al_nm = pp.tile([128, NT], F32, tag="al_nm")
            al_hi_nm = pp.tile([128, NT], F16, tag="al_hi")
            al_lo_nm = pp.tile([128, NT], F16, tag="al_lo")

            nc.sync.dma_start(out=dstloc_sb[:], in_=dstloc_d[:])
            nc.sync.dma_start(out=dinv_sb[:], in_=dinv_d[:])
            for i in range(4):
                nc.sync.dma_start(out=atts[i][:], in_=att_d[i])
            nc.sync.dma_start(out=b_in_sb[:], in_=b_in_d[:])
            nc.sync.dma_start(out=w_cls_sb[:], in_=w_cls_d[:])
            nc.sync.dma_start(out=b_cls_sb[:], in_=b_cls_d[:])
            nc.gpsimd.iota(iota_sb[:], pattern=[[1, 128]], base=0,
                           channel_multiplier=0,
                           allow_small_or_imprecise_dtypes=True)
            make_identity(nc, ident[:])

            h0e_d = dp.tile([PCP, H], F32, tag="h0e")      # eps * h0
            arf_d = dp.tile([1, PCP], F32, tag="arf")      # ar row-major
            shard1 = dp.tile([PCP, RW], F16, tag="shard1")
            table1 = dp.tile([GN, RW], F16, tag="table1")
            shard2 = dp.tile([PCP, RW], F16, tag="shard2")
            table2 = dp.tile([GN, RW], F16, tag="table2")

            # ---------- Phase A ----------
            with tc.tile_pool(name="pha", bufs=3) as pa, \
                 tc.tile_pool(name="phaps", bufs=2, space="PSUM") as paps, \
                 tc.tile_pool(name="phaw", bufs=1) as paw:
                ws = [paw.tile([128, H], F32, tag=f"w{c}", name=f"w{c}")
                      for c in range(4)]
                for c in range(4):
                    nc.sync.dma_start(out=ws[c][:], in_=w_in[c])
                cols_list = [(j * 512, 512) for j in range(PCP // 512)]
                if PCP % 512:
                    cols_list.append((PCP - PCP % 512, PCP % 512))
                for (c0, cw) in cols_list:
                    ps = paps.tile([128, 512], F32, tag="hps")
                    xts = []
                    for c in range(4):
                        xc = pa.tile([128, 512], F32, tag=f"x{c}", name=f"xc{c}")
                        nc.sync.dma_start(out=xc[:, :cw], in_=xt[c, :, c0:c0 + cw])
                        xts.append(xc)
                    for c in range(4):
                        nc.tensor.matmul(out=ps[:, :cw], lhsT=ws[c][:],
                                         rhs=xts[c][:, :cw],
                                         start=(c == 0), stop=(c == 3))
                    hT = pa.tile([128, 512], F32, tag="hT")
                    nc.vector.tensor_scalar(out=hT[:, :cw], in0=ps[:, :cw],
                                            scalar1=b_in_sb[:], scalar2=None,
                                            op0=ALU.add)
                    for bblk in range(cw // 128):
                        t_glob = (c0 + bblk * 128) // 128
                        pt = paps.tile([128, 128], F32, tag="tps")
                        nc.tensor.transpose(
                            out=pt[:], in_=hT[:, bblk * 128:(bblk + 1) * 128],
                            identity=ident[:])
                        nc.vector.tensor_copy(out=cur_nm[:, t_glob, :], in_=pt[:])
                        h0e = pa.tile([128, H], F32, tag="h0e")
                        nc.vector.tensor_scalar(out=h0e[:],
                                                in0=cur_nm[:, t_glob, :],
                                                scalar1=EPS, scalar2=None,
                                                op0=ALU.mult)
                        nc.sync.dma_start(
                            out=h0e_d[t_glob * 128:(t_glob + 1) * 128, :],
                            in_=h0e[:])

            def rowdot(dst_tile_col, att_t, pool, t0, tw):
                tmp = pool.tile([128, 16, H], F32, tag="rd_tmp")
                nc.vector.tensor_tensor(
                    out=tmp[:, :tw, :], in0=cur_nm[:, t0:t0 + tw, :],
                    in1=att_t[:].rearrange("p (o f) -> p o f", o=1)
                    .to_broadcast([128, tw, H]),
                    op=ALU.mult)
                nc.vector.tensor_reduce(
                    out=dst_tile_col[:, t0:t0 + tw], in_=tmp[:, :tw, :],
                    axis=mybir.AxisListType.X, op=ALU.add)

            def build_layer(layer, shard_t, table_t, al_i, ar_i):
                with tc.tile_pool(name=f"rd{layer}", bufs=2) as rp:
                    for t0 in range(0, NT, 16):
                        tw = min(16, NT - t0)
                        rowdot(al_nm, atts[al_i], rp, t0, tw)
                        rowdot(ar_nm, atts[ar_i], rp, t0, tw)
                # ar -> row-major DRAM (for per-dst-tile broadcast loads)
                nc.sync.dma_start(
                    out=arf_d[:].rearrange("o (t p) -> p t o", p=128),
                    in_=ar_nm[:].rearrange("p (t o) -> p t o", o=1))
                # al hi/lo split (fp16 pair)
                nc.vector.tensor_copy(out=al_hi_nm[:], in_=al_nm[:])
                nc.vector.tensor_tensor(out=al_lo_nm[:], in0=al_nm[:],
                                        in1=al_hi_nm[:], op=ALU.subtract)
                # table shard: [dinv*h hi | al hi | al lo | dinv*h lo(0:126)]
                with tc.tile_pool(name=f"asm{layer}", bufs=3) as ap_:
                    for t in range(NT):
                        asm = ap_.tile([128, RW], F16, tag="asm")
                        nc.vector.tensor_scalar(
                            out=asm[:, 0:H], in0=cur_nm[:, t, :],
                            scalar1=dinv_sb[:, t:t + 1], scalar2=None,
                            op0=ALU.mult)
                        nc.vector.scalar_tensor_tensor(
                            out=asm[:, H + 2:RW], in0=cur_nm[:, t, 0:126],
                            scalar=dinv_sb[:, t:t + 1],
                            in1=asm[:, 0:126], op0=ALU.mult, op1=ALU.subtract)
                        nc.vector.tensor_copy(out=asm[:, H:H + 1],
                                              in_=al_hi_nm[:, t:t + 1])
                        nc.vector.tensor_copy(out=asm[:, H + 1:H + 2],
                                              in_=al_lo_nm[:, t:t + 1])
                        nc.sync.dma_start(out=shard_t[t * 128:(t + 1) * 128, :],
                                          in_=asm[:])
                nc.gpsimd.collective_compute(
                    "AllGather", ALU.bypass,
                    replica_groups=[list(range(NC_))],
                    ins=[shard_t.opt()], outs=[table_t.opt()])

                with tc.tile_pool(name=f"ed{layer}", bufs=3) as ep, \
                     tc.tile_pool(name=f"edm{layer}", bufs=2) as mp, \
                     tc.tile_pool(name=f"edps{layer}", bufs=2,
                                  space="PSUM") as pps, \
                     tc.tile_pool(name=f"edix{layer}", bufs=4) as ixp, \
                     tc.tile_pool(name=f"ev{layer}", bufs=3) as vp, \
                     tc.tile_pool(name=f"arr{layer}", bufs=2 * SUP) as arp:

                    def evac(s, psums):
                        sup_w = min(SUP, NT - s * SUP)
                        for ti in range(sup_w):
                            t_glob = s * SUP + ti
                            ps = psums[ti]
                            h0t = vp.tile([128, H], F32, tag="h0t")
                            nc.sync.dma_start(
                                out=h0t[:],
                                in_=h0e_d[t_glob * 128:(t_glob + 1) * 128, :])
                            if layer == 1:
                                tmp = vp.tile([128, H], F32, tag="ev")
                                nc.vector.scalar_tensor_tensor(
                                    out=tmp[:], in0=ps[:],
                                    scalar=dinv_sb[:, t_glob:t_glob + 1],
                                    in1=h0t[:], op0=ALU.mult, op1=ALU.add)
                                nc.scalar.activation(out=cur_nm[:, t_glob, :],
                                                     in_=tmp[:], func=AF.Relu)
                            else:
                                nc.vector.scalar_tensor_tensor(
                                    out=cur_nm[:, t_glob, :], in0=ps[:],
                                    scalar=dinv_sb[:, t_glob:t_glob + 1],
                                    in1=h0t[:], op0=ALU.mult, op1=ALU.add)

                    s_cur = -1
                    psums = {}
                    started = {}
                    arreps = {}
                    qn = 0
                    for ci, (s, b_, toff, tl, ioff, nidx) in enumerate(calls):
                        if s != s_cur:
                            if s_cur >= 0:
                                evac(s_cur, psums)
                            psums = {}
                            started = {}
                            arreps = {}
                            s_cur = s
                            for ti in range(len(tl)):
                                psums[ti] = pps.tile(
                                    [128, H], F32, tag=f"ps{ti}",
                                    name=f"ps_{layer}_{s}_{ti}")
                                t_glob = s * SUP + ti
                                arr = arp.tile([128, 128], F32, tag=f"ar{ti}",
                                               name=f"ar_{layer}_{s}_{ti}")
                                nc.sync.dma_start(
                                    out=arr[:],
                                    in_=arf_d[0:1,
                                              t_glob * 128:(t_glob + 1) * 128]
                                    .to_broadcast([128, 128]))
                                arreps[ti] = arr
                        ntl = sum(tl)
                        ixt = ixp.tile([128, nidx // 16], I16, tag="ix")
                        nc.sync.dma_start(out=ixt[:],
                                          in_=gidx_d[:, ioff:ioff + nidx // 16])
                        tg = ep.tile([128, ntl, RW], F16, tag="gat")
                        nc.gpsimd.dma_gather(
                            tg[:], table_t[b_ * BROWS:(b_ + 1) * BROWS, :],
                            ixt[:], nidx, nidx, RW,
                            single_packet=False, queue_num=qn)
                        qn = (qn + 1) % 4
                        # f32 messages: hi + lo (features 0..125), hi (126..7)
                        msg = mp.tile([128, ntl, H], F32, tag="msg")
                        nc.vector.tensor_tensor(
                            out=msg[:, :, 0:126], in0=tg[:, :, 0:126],
                            in1=tg[:, :, H + 2:RW], op=ALU.add)
                        nc.vector.tensor_copy(out=msg[:, :, 126:128],
                                              in_=tg[:, :, 126:128])
                        # al_s (f32) per edge
                        alv = mp.tile([128, ntl], F32, tag="alv")
                        nc.vector.tensor_tensor(
                            out=alv[:].rearrange("p (n o) -> p n o", o=1),
                            in0=tg[:, :, H:H + 1], in1=tg[:, :, H + 1:H + 2],
                            op=ALU.add)
                        # Z = ar_row + al_s ; tanh on ScalarE
                        zt = mp.tile([128, ntl, 128], F32, tag="zt")
                        r0 = 0
                        for ti, w in enumerate(tl):
                            if w == 0:
                                continue
                            nc.vector.tensor_tensor(
                                out=zt[:, r0:r0 + w, :],
                                in0=arreps[ti][:]
                                .rearrange("p (o f) -> p o f", o=1)
                                .to_broadcast([128, w, 128]),
                                in1=alv[:, r0:r0 + w]
                                .to_broadcast([128, w, 128]),
                                op=ALU.add)
                            r0 += w
                        nc.scalar.activation(out=zt[:], in_=zt[:], func=AF.Tanh)
                        # M = onehot(dstloc) * tanh(Z)
                        mb = mp.tile([128, ntl, 128], F32, tag="mb")
                        nc.vector.tensor_tensor(
                            out=mb[:],
                            in0=dstloc_sb[:, toff:toff + ntl]
                            .to_broadcast([128, ntl, 128]),
                            in1=iota_sb[:].rearrange("p (o f) -> p o f", o=1)
                            .to_broadcast([128, ntl, 128]),
                            op=ALU.is_equal)
                        nc.vector.tensor_tensor(
                            out=mb[:], in0=mb[:], in1=zt[:], op=ALU.mult)
                        r0 = 0
                        for ti, w in enumerate(tl):
                            for j in range(w):
                                first = not started.get(ti, False)
                                started[ti] = True
                                stop = last_mm.get((s, ti)) == (ci, ti, j)
                                nc.tensor.matmul(
                                    out=psums[ti][:],
                                    lhsT=mb[:, r0 + j, :],
                                    rhs=msg[:, r0 + j, :],
                                    start=first, stop=stop)
                            r0 += w
                    if s_cur >= 0:
                        evac(s_cur, psums)

            build_layer(1, shard1, table1, 0, 1)
            build_layer(2, shard2, table2, 2, 3)

            # ---------- Phase D: head ----------
            with tc.tile_pool(name="hd", bufs=2) as hp, \
                 tc.tile_pool(name="hdps", bufs=4, space="PSUM") as hps:
                for t0 in range(0, NT, 14):
                    tw = min(14, NT - t0)
                    lg = hp.tile([128, 14, C], F32, tag="lg")
                    for ti in range(tw):
                        t = t0 + ti
                        tp_ = hps.tile([128, H], F32, tag="tp")
                        nc.tensor.transpose(out=tp_[:], in_=cur_nm[:, t, :],
                                            identity=ident[:])
                        h2T = hp.tile([128, H], F32, tag="h2T")
                        nc.vector.tensor_copy(out=h2T[:], in_=tp_[:])
                        lp = hps.tile([128, C], F32, tag="lp")
                        nc.tensor.matmul(out=lp[:], lhsT=h2T[:],
                                         rhs=w_cls_sb[:], start=True, stop=True)
                        nc.vector.tensor_tensor(out=lg[:, ti, :], in0=lp[:],
                                                in1=b_cls_sb[:], op=ALU.add)
                        nc.sync.dma_start(out=emb_d[t * 128:(t + 1) * 128, :],
                                          in_=cur_nm[:, t, :])
                    nc.sync.dma_start(
                        out=logits_d.ap().rearrange("(t p) c -> p t c", p=128)
                        [:, t0:t0 + tw, :],
                        in_=lg[:, :tw, :])
                    mx = hp.tile([128, 14], F32, tag="mx")
                    nc.vector.tensor_reduce(out=mx[:, :tw], in_=lg[:, :tw, :],
                                            axis=mybir.AxisListType.X,
                                            op=ALU.max)
                    ex = hp.tile([128, 14, C], F32, tag="ex")
                    nc.vector.tensor_tensor(
                        out=ex[:, :tw, :], in0=lg[:, :tw, :],
                        in1=mx[:, :tw].to_broadcast([128, tw, C]),
                        op=ALU.subtract)
                    nc.scalar.activation(out=ex[:, :tw, :], in_=ex[:, :tw, :],
                                         func=AF.Exp)
                    sm = hp.tile([128, 14], F32, tag="sm")
                    nc.vector.tensor_reduce(out=sm[:, :tw], in_=ex[:, :tw, :],
                                            axis=mybir.AxisListType.X,
                                            op=ALU.add)
                    rc = hp.tile([128, 14], F32, tag="rc")
                    nc.vector.reciprocal(out=rc[:, :tw], in_=sm[:, :tw])
                    nc.vector.tensor_tensor(
                        out=ex[:, :tw, :], in0=ex[:, :tw, :],
                        in1=rc[:, :tw].to_broadcast([128, tw, C]),
                        op=ALU.mult)
                    nc.sync.dma_start(
                        out=soft_d.ap().rearrange("(t p) c -> p t c", p=128)
                        [:, t0:t0 + tw, :],
                        in_=ex[:, :tw, :])
                    hmx = hp.tile([128, 14, 8], F32, tag="hmx")
                    hix = hp.tile([128, 14, 8], U32, tag="hix")
                    for ti in range(tw):
                        nc.vector.max(out=hmx[:, ti, :], in_=lg[:, ti, :])
                        nc.vector.max_index(out=hix[:, ti, :],
                                            in_max=hmx[:, ti, :],
                                            in_values=lg[:, ti, :])
                    nc.sync.dma_start(
                        out=hard_d.ap().rearrange("(t p) c -> p t c", p=128)
                        [:, t0:t0 + tw, :],
                        in_=hix[:, :tw, :])
    nc.compile()
    return nc


def kernel(x, edge_index, W_in, b_in, att_l1, att_r1, att_l2, att_r2,
           W_cls, b_cls):
    global LAST_EXEC_NS
    x = np.asarray(x)
    edge_index = np.asarray(edge_index)
    meta, per_core = _prep_edges(edge_index)
    nc = _build_nc(meta)

    w_in_p = np.zeros((4, 128, H), np.float32)
    w_in_p.reshape(512, H)[:IN] = np.asarray(W_in, np.float32)
    b_in_p = np.asarray(b_in, np.float32).reshape(128, 1)
    att_p = np.stack([np.tile(np.asarray(a, np.float32)[None, :], (128, 1))
                      for a in (att_l1, att_r1, att_l2, att_r2)])
    w_cls_p = np.asarray(W_cls, np.float32)
    b_cls_p = np.tile(np.asarray(b_cls, np.float32)[None, :], (128, 1))

    in_maps = []
    for k in range(NC_):
        xk = np.zeros((PCP, INP), np.float32)
        xk[:PC, :IN] = x[k * PC:(k + 1) * PC]
        xt = np.ascontiguousarray(xk.T).reshape(4, 128, PCP)
        in_maps.append(dict(
            xt=xt, w_in=w_in_p, b_in=b_in_p, att=att_p,
            w_cls=w_cls_p, b_cls=b_cls_p,
            gidx=per_core[k]["gidx"], dstloc=per_core[k]["dstloc"],
            dinv=per_core[k]["dinv"]))

    trace = _enable_trace()
    import tempfile
    res = run_bass_kernel_spmd(nc, in_maps, core_ids=list(range(NC_)),
                               trace=trace, tmpdir=tempfile.mkdtemp())
    LAST_EXEC_NS = res.exec_time_ns

    logits = np.zeros((N, C), np.float32)
    emb = np.zeros((N, H), np.float32)
    soft = np.zeros((N, C), np.float32)
    hard = np.zeros((N,), np.int32)
    for k in range(NC_):
        r = res.results[k]
        logits[k * PC:(k + 1) * PC] = r["logits"][:PC]
        emb[k * PC:(k + 1) * PC] = r["emb"][:PC]
        soft[k * PC:(k + 1) * PC] = r["soft"][:PC]
        hard[k * PC:(k + 1) * PC] = r["hard"][:PC, 0].astype(np.int32)
    return logits, emb, soft, hard


# revision 8
# speedup vs baseline: 1.0675x; 1.0675x over previous
"""FAGCN (2-layer FAConv GNN) Trainium2 kernel, 8 NeuronCores SPMD.

Sharding: nodes by id-range across 8 cores (12500 each); edges partitioned by
dst so segment-sum is local; per-layer halo exchange = AllGather of the
per-node table; small weights replicated.

Table row (fp16 x 256 = 512B): [dinv*h hi (128) | al hi | al lo | dinv*h lo
(126, features 0..125)] - hi+lo fp16 pairs carry ~22-bit mantissa (~f32).
dinv_src is folded into the table rows and dinv_dst into the PSUM evacuation,
so the per-edge coefficient is just tanh(al_src + ar_dst).

Per-core pipeline:
  A) h = x @ W_in + b_in (f32 PE matmuls from host-transposed x), h0e=eps*h
  B) per layer: al/ar row-dots, build table shard -> AllGather -> full table
  C) per layer, per (supertile x src-bank) call: dma_gather 512B rows by edge
     src (int16 bank-local ids), reconstruct f32 messages, Z = ar_row + al_s,
     tanh on ScalarE, M = onehot(dstloc) * tanh(Z), f32 one-hot matmul
     segment-sum into PSUM, evac dinv_d*agg + eps*h0 (+relu for layer 1)
  D) logits / softmax / argmax head
"""
import sys
import numpy as np

for _p in ('/opt/trn_rl_repo', '/root/.axon_site'):
    if _p not in sys.path:
        sys.path.insert(0, _p)

from concourse import bass, mybir  # noqa: E402
import concourse.tile as tile  # noqa: E402
from concourse import bacc  # noqa: E402
from concourse.masks import make_identity  # noqa: E402
from concourse.bass_utils import run_bass_kernel_spmd  # noqa: E402

F32 = mybir.dt.float32
F16 = mybir.dt.float16
I16 = mybir.dt.int16
U32 = mybir.dt.uint32
AF = mybir.ActivationFunctionType
ALU = mybir.AluOpType

N, E, IN, H, C = 100000, 1600000, 500, 128, 40
EPS = 0.1
NC_ = 8                     # cores
PC = 12500                  # real nodes per core
NT = 98                     # dst tiles per core
PCP = NT * 128              # 12544 padded nodes per core
GN = NC_ * PCP              # 100352 padded global rows
NBANK = 4
BROWS = GN // NBANK         # 25088 rows per bank (< 32768 for int16 idx)
SUP = 4                     # dst tiles per supertile
RW = 256                    # fp16 elems per table row (512B)
INP = 512                   # padded input dim

LAST_EXEC_NS = None


def _enable_trace():
    try:
        import types
        import antenv
        if 'antenv.axon_hooks' not in sys.modules:
            hm = types.ModuleType('antenv.axon_hooks')
            _h = {}
            hm.set_axon_ntff_profile_hook = lambda h: _h.__setitem__('h', h)
            hm.get_axon_ntff_profile_hook = lambda: _h.get('h')
            sys.modules['antenv.axon_hooks'] = hm
            antenv.axon_hooks = hm
            from trn_agent_boot.trn_boot import _ntff_profile_via_ctypes
            hook = _ntff_profile_via_ctypes('/opt/axon/libaxon_pjrt.so')
            if hook is not None:
                hm.set_axon_ntff_profile_hook(hook)
        return sys.modules['antenv.axon_hooks'].get_axon_ntff_profile_hook() is not None
    except Exception:
        return False


def _wrap_idxs(idx):
    n = len(idx)
    S = -(-n // 16)
    flat = np.zeros(S * 16, np.int64)
    flat[:n] = idx
    buf = flat.reshape(S, 16).T.astype(np.int16)
    return np.tile(buf, (8, 1))


def _prep_edges(edge_index):
    src = np.concatenate([edge_index[0], np.arange(N, dtype=np.int64)])
    dst = np.concatenate([edge_index[1], np.arange(N, dtype=np.int64)])
    deg = np.bincount(dst, minlength=N).astype(np.float64)
    dinv = (1.0 / np.sqrt(deg)).astype(np.float32)
    row = (src // PC) * PCP + (src % PC)
    core = dst // PC

    NS = (NT + SUP - 1) // SUP
    counts = np.zeros((NC_, NS, NBANK, SUP), np.int64)
    buckets = [[[[None] * SUP for _ in range(NBANK)] for _ in range(NS)]
               for _ in range(NC_)]
    for k in range(NC_):
        m = core == k
        er, ed = row[m], (dst[m] - k * PC).astype(np.int64)
        t = ed >> 7
        b = er // BROWS
        key = (t // SUP) * (NBANK * SUP) + b * SUP + (t % SUP)
        order = np.argsort(key, kind='stable')
        er, ed, key = er[order], ed[order], key[order]
        bnd = np.searchsorted(key, np.arange(NS * NBANK * SUP + 1))
        for s in range(NS):
            for b_ in range(NBANK):
                for ti in range(SUP):
                    kk = s * (NBANK * SUP) + b_ * SUP + ti
                    lo, hi = bnd[kk], bnd[kk + 1]
                    counts[k, s, b_, ti] = hi - lo
                    buckets[k][s][b_][ti] = (er[lo:hi] - b_ * BROWS,
                                             ed[lo:hi] - (s * SUP + ti) * 128)
    mx = counts.max(axis=0)
    ntile = -(-mx // 128)
    calls = []
    tile_off = 0
    idx_coloff = 0
    for s in range(NS):
        sup_w = min(SUP, NT - s * SUP)
        for b_ in range(NBANK):
            tl = [int(ntile[s, b_, ti]) for ti in range(sup_w)]
            ntl = sum(tl)
            if ntl == 0:
                continue
            calls.append((s, b_, tile_off, tl, idx_coloff, ntl * 128))
            tile_off += ntl
            idx_coloff += ntl * 8
    TT, GW = tile_off, idx_coloff
    per_core = []
    for k in range(NC_):
        gidx = np.zeros((128, GW), np.int16)
        dstloc = np.zeros((128, TT), np.float32)
        for (s, b_, toff, tl, ioff, nidx) in calls:
            ii = np.zeros(nidx, np.int64)
            dd = np.full(nidx, -1.0, np.float64)   # pad edges match no column
            pos = 0
            for ti, ntl_t in enumerate(tl):
                er, ed = buckets[k][s][b_][ti]
                ii[pos:pos + len(er)] = er
                dd[pos:pos + len(ed)] = ed
                pos += ntl_t * 128
            gidx[:, ioff:ioff + nidx // 16] = _wrap_idxs(ii)
            dstloc[:, toff:toff + nidx // 128] = dd.reshape(-1, 128).T
        per_core.append(dict(gidx=gidx, dstloc=dstloc,
                             dinv=np.ascontiguousarray(
                                 np.pad(dinv[k * PC:(k + 1) * PC],
                                        (0, PCP - PC), constant_values=1.0)
                                 .reshape(NT, 128).T)))
    return dict(calls=calls, TT=TT, GW=GW, NS=NS), per_core


def _build_nc(meta):
    nc = bacc.Bacc("TRN2", target_bir_lowering=False, debug=False,
                   num_devices=NC_, num_swdge_queues=4)
    xt = nc.dram_tensor("xt", [4, 128, PCP], F32, kind="ExternalInput")
    w_in = nc.dram_tensor("w_in", [4, 128, H], F32, kind="ExternalInput")
    b_in_d = nc.dram_tensor("b_in", [128, 1], F32, kind="ExternalInput")
    att_d = nc.dram_tensor("att", [4, 128, H], F32, kind="ExternalInput")
    w_cls_d = nc.dram_tensor("w_cls", [128, C], F32, kind="ExternalInput")
    b_cls_d = nc.dram_tensor("b_cls", [128, C], F32, kind="ExternalInput")
    gidx_d = nc.dram_tensor("gidx", [128, meta["GW"]], I16, kind="ExternalInput")
    dstloc_d = nc.dram_tensor("dstloc", [128, meta["TT"]], F32,
                              kind="ExternalInput")
    dinv_d = nc.dram_tensor("dinv", [128, NT], F32, kind="ExternalInput")
    logits_d = nc.dram_tensor("logits", [PCP, C], F32, kind="ExternalOutput")
    emb_d = nc.dram_tensor("emb", [PCP, H], F32, kind="ExternalOutput")
    soft_d = nc.dram_tensor("soft", [PCP, C], F32, kind="ExternalOutput")
    hard_d = nc.dram_tensor("hard", [PCP, 8], U32, kind="ExternalOutput")

    calls = meta["calls"]
    last_mm = {}
    for ci, (s, b_, toff, tl, ioff, nidx) in enumerate(calls):
        for ti, w in enumerate(tl):
            if w > 0:
                last_mm[(s, ti)] = (ci, ti, w - 1)

    with tile.TileContext(nc) as tc:
        with tc.tile_pool(name="persist", bufs=1) as pp, \
             tc.tile_pool(name="dram", bufs=1, space="DRAM") as dp:
            cur_nm = pp.tile([128, NT, H], F32, tag="cur")
            dstloc_sb = pp.tile([128, meta["TT"]], F32, tag="dstloc")
            dinv_sb = pp.tile([128, NT], F32, tag="dinv")
            iota_sb = pp.tile([128, 128], F32, tag="iota")
            ident = pp.tile([128, 128], F32, tag="ident")
            atts = [pp.tile([128, H], F32, tag=f"att{i}", name=f"att{i}")
                    for i in range(4)]
            b_in_sb = pp.tile([128, 1], F32, tag="b_in")
            w_cls_sb = pp.tile([128, C], F32, tag="w_cls")
            b_cls_sb = pp.tile([128, C], F32, tag="b_cls")
            ar_nm = pp.tile([128, NT], F32, tag="ar_nm")
            al_nm = pp.tile([128, NT], F32, tag="al_nm")
            al_hi_nm = pp.tile([128, NT], F16, tag="al_hi")
            al_lo_nm = pp.tile([128, NT], F16, tag="al_lo")

            nc.sync.dma_start(out=dstloc_sb[:], in_=dstloc_d[:])
            nc.sync.dma_start(out=dinv_sb[:], in_=dinv_d[:])
            for i in range(4):
                nc.sync.dma_start(out=atts[i][:], in_=att_d[i])
            nc.sync.dma_start(out=b_in_sb[:], in_=b_in_d[:])
            nc.sync.dma_start(out=w_cls_sb[:], in_=w_cls_d[:])
            nc.sync.dma_start(out=b_cls_sb[:], in_=b_cls_d[:])
            nc.gpsimd.iota(iota_sb[:], pattern=[[1, 128]], base=0,
                           channel_multiplier=0,
                           allow_small_or_imprecise_dtypes=True)
            make_identity(nc, ident[:])

            h0e_d = dp.tile([PCP, H], F32, tag="h0e")      # eps * h0
            arf_d = dp.tile([1, PCP], F32, tag="arf")      # ar row-major
            shard1 = dp.tile([PCP, RW], F16, tag="shard1")
            table1 = dp.tile([GN, RW], F16, tag="table1")
            shard2 = dp.tile([PCP, RW], F16, tag="shard2")
            table2 = dp.tile([GN, RW], F16, tag="table2")

            # ---------- Phase A ----------
            with tc.tile_pool(name="pha", bufs=3) as pa, \
                 tc.tile_pool(name="phaps", bufs=2, space="PSUM") as paps, \
                 tc.tile_pool(name="phaw", bufs=1) as paw:
                ws = [paw.tile([128, H], F32, tag=f"w{c}", name=f"w{c}")
                      for c in range(4)]
                for c in range(4):
                    nc.sync.dma_start(out=ws[c][:], in_=w_in[c])
                cols_list = [(j * 512, 512) for j in range(PCP // 512)]
                if PCP % 512:
                    cols_list.append((PCP - PCP % 512, PCP % 512))
                for (c0, cw) in cols_list:
                    ps = paps.tile([128, 512], F32, tag="hps")
                    xts = []
                    for c in range(4):
                        xc = pa.tile([128, 512], F32, tag=f"x{c}", name=f"xc{c}")
                        nc.sync.dma_start(out=xc[:, :cw], in_=xt[c, :, c0:c0 + cw])
                        xts.append(xc)
                    for c in range(4):
                        nc.tensor.matmul(out=ps[:, :cw], lhsT=ws[c][:],
                                         rhs=xts[c][:, :cw],
                                         start=(c == 0), stop=(c == 3))
                    hT = pa.tile([128, 512], F32, tag="hT")
                    nc.vector.tensor_scalar(out=hT[:, :cw], in0=ps[:, :cw],
                                            scalar1=b_in_sb[:], scalar2=None,
                                            op0=ALU.add)
                    for bblk in range(cw // 128):
                        t_glob = (c0 + bblk * 128) // 128
                        pt = paps.tile([128, 128], F32, tag="tps")
                        nc.tensor.transpose(
                            out=pt[:], in_=hT[:, bblk * 128:(bblk + 1) * 128],
                            identity=ident[:])
                        nc.vector.tensor_copy(out=cur_nm[:, t_glob, :], in_=pt[:])
                        h0e = pa.tile([128, H], F32, tag="h0e")
                        nc.vector.tensor_scalar(out=h0e[:],
                                                in0=cur_nm[:, t_glob, :],
                                                scalar1=EPS, scalar2=None,
                                                op0=ALU.mult)
                        nc.sync.dma_start(
                            out=h0e_d[t_glob * 128:(t_glob + 1) * 128, :],
                            in_=h0e[:])

            def rowdot(dst_tile_col, att_t, pool, t0, tw):
                tmp = pool.tile([128, 16, H], F32, tag="rd_tmp")
                nc.vector.tensor_tensor(
                    out=tmp[:, :tw, :], in0=cur_nm[:, t0:t0 + tw, :],
                    in1=att_t[:].rearrange("p (o f) -> p o f", o=1)
                    .to_broadcast([128, tw, H]),
                    op=ALU.mult)
                nc.vector.tensor_reduce(
                    out=dst_tile_col[:, t0:t0 + tw], in_=tmp[:, :tw, :],
                    axis=mybir.AxisListType.X, op=ALU.add)

            def build_layer(layer, shard_t, table_t, al_i, ar_i):
                with tc.tile_pool(name=f"rd{layer}", bufs=2) as rp:
                    for t0 in range(0, NT, 16):
                        tw = min(16, NT - t0)
                        rowdot(al_nm, atts[al_i], rp, t0, tw)
                        rowdot(ar_nm, atts[ar_i], rp, t0, tw)
                # ar -> row-major DRAM (for per-dst-tile broadcast loads)
                nc.sync.dma_start(
                    out=arf_d[:].rearrange("o (t p) -> p t o", p=128),
                    in_=ar_nm[:].rearrange("p (t o) -> p t o", o=1))
                # al hi/lo split (fp16 pair)
                nc.vector.tensor_copy(out=al_hi_nm[:], in_=al_nm[:])
                nc.vector.tensor_tensor(out=al_lo_nm[:], in0=al_nm[:],
                                        in1=al_hi_nm[:], op=ALU.subtract)
                # table shard: [dinv*h hi | al hi | al lo | dinv*h lo(0:126)]
                with tc.tile_pool(name=f"asm{layer}", bufs=3) as ap_:
                    for t in range(NT):
                        asm = ap_.tile([128, RW], F16, tag="asm")
                        nc.vector.tensor_scalar(
                            out=asm[:, 0:H], in0=cur_nm[:, t, :],
                            scalar1=dinv_sb[:, t:t + 1], scalar2=None,
                            op0=ALU.mult)
                        nc.vector.scalar_tensor_tensor(
                            out=asm[:, H + 2:RW], in0=cur_nm[:, t, 0:126],
                            scalar=dinv_sb[:, t:t + 1],
                            in1=asm[:, 0:126], op0=ALU.mult, op1=ALU.subtract)
                        nc.vector.tensor_copy(out=asm[:, H:H + 1],
                                              in_=al_hi_nm[:, t:t + 1])
                        nc.vector.tensor_copy(out=asm[:, H + 1:H + 2],
                                              in_=al_lo_nm[:, t:t + 1])
                        nc.sync.dma_start(out=shard_t[t * 128:(t + 1) * 128, :],
                                          in_=asm[:])
                nc.gpsimd.collective_compute(
                    "AllGather", ALU.bypass,
                    replica_groups=[list(range(NC_))],
                    ins=[shard_t.opt()], outs=[table_t.opt()])

                with tc.tile_pool(name=f"ed{layer}", bufs=4) as ep, \
                     tc.tile_pool(name=f"edm{layer}", bufs=4) as mp, \
                     tc.tile_pool(name=f"edps{layer}", bufs=2,
                                  space="PSUM") as pps, \
                     tc.tile_pool(name=f"edix{layer}", bufs=4) as ixp, \
                     tc.tile_pool(name=f"ev{layer}", bufs=3) as vp, \
                     tc.tile_pool(name=f"arr{layer}", bufs=2 * SUP) as arp:

                    def evac(s, psums):
                        sup_w = min(SUP, NT - s * SUP)
                        for ti in range(sup_w):
                            t_glob = s * SUP + ti
                            ps = psums[ti]
                            h0t = vp.tile([128, H], F32, tag="h0t")
                            nc.sync.dma_start(
                                out=h0t[:],
                                in_=h0e_d[t_glob * 128:(t_glob + 1) * 128, :])
                            if layer == 1:
                                tmp = vp.tile([128, H], F32, tag="ev")
                                nc.vector.scalar_tensor_tensor(
                                    out=tmp[:], in0=ps[:],
                                    scalar=dinv_sb[:, t_glob:t_glob + 1],
                                    in1=h0t[:], op0=ALU.mult, op1=ALU.add)
                                nc.scalar.activation(out=cur_nm[:, t_glob, :],
                                                     in_=tmp[:], func=AF.Relu)
                            else:
                                nc.vector.scalar_tensor_tensor(
                                    out=cur_nm[:, t_glob, :], in0=ps[:],
                                    scalar=dinv_sb[:, t_glob:t_glob + 1],
                                    in1=h0t[:], op0=ALU.mult, op1=ALU.add)

                    s_cur = -1
                    psums = {}
                    started = {}
                    arreps = {}
                    qn = 0
                    for ci, (s, b_, toff, tl, ioff, nidx) in enumerate(calls):
                        if s != s_cur:
                            if s_cur >= 0:
                                evac(s_cur, psums)
                            psums = {}
                            started = {}
                            arreps = {}
                            s_cur = s
                            for ti in range(len(tl)):
                                psums[ti] = pps.tile(
                                    [128, H], F32, tag=f"ps{ti}",
                                    name=f"ps_{layer}_{s}_{ti}")
                                t_glob = s * SUP + ti
                                arr = arp.tile([128, 128], F32, tag=f"ar{ti}",
                                               name=f"ar_{layer}_{s}_{ti}")
                                nc.sync.dma_start(
                                    out=arr[:],
                                    in_=arf_d[0:1,
                                              t_glob * 128:(t_glob + 1) * 128]
                                    .to_broadcast([128, 128]))
                                arreps[ti] = arr
                        ntl = sum(tl)
                        ixt = ixp.tile([128, nidx // 16], I16, tag="ix")
                        nc.sync.dma_start(out=ixt[:],
                                          in_=gidx_d[:, ioff:ioff + nidx // 16])
                        tg = ep.tile([128, ntl, RW], F16, tag="gat")
                        nc.gpsimd.dma_gather(
                            tg[:], table_t[b_ * BROWS:(b_ + 1) * BROWS, :],
                            ixt[:], nidx, nidx, RW,
                            single_packet=False, queue_num=qn)
                        qn = (qn + 1) % 4
                        # f32 messages: hi + lo (features 0..125), hi (126..7)
                        msg = mp.tile([128, ntl, H], F32, tag="msg")
                        nc.vector.tensor_tensor(
                            out=msg[:, :, 0:126], in0=tg[:, :, 0:126],
                            in1=tg[:, :, H + 2:RW], op=ALU.add)
                        nc.vector.tensor_copy(out=msg[:, :, 126:128],
                                              in_=tg[:, :, 126:128])
                        # al_s (f32) per edge
                        alv = mp.tile([128, ntl], F32, tag="alv")
                        nc.vector.tensor_tensor(
                            out=alv[:].rearrange("p (n o) -> p n o", o=1),
                            in0=tg[:, :, H:H + 1], in1=tg[:, :, H + 1:H + 2],
                            op=ALU.add)
                        # per tile: tanh(ar_row + al_s) on ScalarE (bias
                        # = per-edge al), then M = onehot(dstloc)*tanh(Z) in
                        # one fused DVE op, then the segment-sum matmul
                        r0 = 0
                        for ti, w in enumerate(tl):
                            for j in range(w):
                                zt = mp.tile([128, 128], F32, tag="zt",
                                             name=f"zt_{layer}_{ci}_{r0+j}")
                                nc.scalar.activation(
                                    out=zt[:], in_=arreps[ti][:],
                                    func=AF.Tanh,
                                    bias=alv[:, r0 + j:r0 + j + 1])
                                mb = mp.tile([128, 128], F32, tag="mb",
                                             name=f"mb_{layer}_{ci}_{r0+j}")
                                nc.vector.scalar_tensor_tensor(
                                    out=mb[:], in0=iota_sb[:],
                                    scalar=dstloc_sb[:,
                                                     toff + r0 + j:
                                                     toff + r0 + j + 1],
                                    in1=zt[:], op0=ALU.is_equal, op1=ALU.mult)
                                first = not started.get(ti, False)
                                started[ti] = True
                                stop = last_mm.get((s, ti)) == (ci, ti, j)
                                nc.tensor.matmul(
                                    out=psums[ti][:],
                                    lhsT=mb[:],
                                    rhs=msg[:, r0 + j, :],
                                    start=first, stop=stop)
                            r0 += w
                    if s_cur >= 0:
                        evac(s_cur, psums)

            build_layer(1, shard1, table1, 0, 1)
            build_layer(2, shard2, table2, 2, 3)

            # ---------- Phase D: head ----------
            with tc.tile_pool(name="hd", bufs=2) as hp, \
                 tc.tile_pool(name="hdps", bufs=4, space="PSUM") as hps:
                for t0 in range(0, NT, 14):
                    tw = min(14, NT - t0)
                    lg = hp.tile([128, 14, C], F32, tag="lg")
                    for ti in range(tw):
                        t = t0 + ti
                        tp_ = hps.tile([128, H], F32, tag="tp")
                        nc.tensor.transpose(out=tp_[:], in_=cur_nm[:, t, :],
                                            identity=ident[:])
                        h2T = hp.tile([128, H], F32, tag="h2T")
                        nc.vector.tensor_copy(out=h2T[:], in_=tp_[:])
                        lp = hps.tile([128, C], F32, tag="lp")
                        nc.tensor.matmul(out=lp[:], lhsT=h2T[:],
                                         rhs=w_cls_sb[:], start=True, stop=True)
                        nc.vector.tensor_tensor(out=lg[:, ti, :], in0=lp[:],
                                                in1=b_cls_sb[:], op=ALU.add)
                        nc.sync.dma_start(out=emb_d[t * 128:(t + 1) * 128, :],
                                          in_=cur_nm[:, t, :])
                    nc.sync.dma_start(
                        out=logits_d.ap().rearrange("(t p) c -> p t c", p=128)
                        [:, t0:t0 + tw, :],
                        in_=lg[:, :tw, :])
                    mx = hp.tile([128, 14], F32, tag="mx")
                    nc.vector.tensor_reduce(out=mx[:, :tw], in_=lg[:, :tw, :],
                                            axis=mybir.AxisListType.X,
                                            op=ALU.max)
                    ex = hp.tile([128, 14, C], F32, tag="ex")
                    nc.vector.tensor_tensor(
                        out=ex[:, :tw, :], in0=lg[:, :tw, :],
                        in1=mx[:, :tw].to_broadcast([128, tw, C]),
                        op=ALU.subtract)
                    nc.scalar.activation(out=ex[:, :tw, :], in_=ex[:, :tw, :],
                                         func=AF.Exp)
                    sm = hp.tile([128, 14], F32, tag="sm")
                    nc.vector.tensor_reduce(out=sm[:, :tw], in_=ex[:, :tw, :],
                                            axis=mybir.AxisListType.X,
                                            op=ALU.add)
                    rc = hp.tile([128, 14], F32, tag="rc")
                    nc.vector.reciprocal(out=rc[:, :tw], in_=sm[:, :tw])
                    nc.vector.tensor_tensor(
                        out=ex[:, :tw, :], in0=ex[:, :tw, :],
                        in1=rc[:, :tw].to_broadcast([128, tw, C]),
                        op=ALU.mult)
                    nc.sync.dma_start(
                        out=soft_d.ap().rearrange("(t p) c -> p t c", p=128)
                        [:, t0:t0 + tw, :],
                        in_=ex[:, :tw, :])
                    hmx = hp.tile([128, 14, 8], F32, tag="hmx")
                    hix = hp.tile([128, 14, 8], U32, tag="hix")
                    for ti in range(tw):
                        nc.vector.max(out=hmx[:, ti, :], in_=lg[:, ti, :])
                        nc.vector.max_index(out=hix[:, ti, :],
                                            in_max=hmx[:, ti, :],
                                            in_values=lg[:, ti, :])
                    nc.sync.dma_start(
                        out=hard_d.ap().rearrange("(t p) c -> p t c", p=128)
                        [:, t0:t0 + tw, :],
                        in_=hix[:, :tw, :])
    nc.compile()
    return nc


def kernel(x, edge_index, W_in, b_in, att_l1, att_r1, att_l2, att_r2,
           W_cls, b_cls):
    global LAST_EXEC_NS
    x = np.asarray(x)
    edge_index = np.asarray(edge_index)
    meta, per_core = _prep_edges(edge_index)
    nc = _build_nc(meta)

    w_in_p = np.zeros((4, 128, H), np.float32)
    w_in_p.reshape(512, H)[:IN] = np.asarray(W_in, np.float32)
    b_in_p = np.asarray(b_in, np.float32).reshape(128, 1)
    att_p = np.stack([np.tile(np.asarray(a, np.float32)[None, :], (128, 1))
                      for a in (att_l1, att_r1, att_l2, att_r2)])
    w_cls_p = np.asarray(W_cls, np.float32)
    b_cls_p = np.tile(np.asarray(b_cls, np.float32)[None, :], (128, 1))

    in_maps = []
    for k in range(NC_):
        xk = np.zeros((PCP, INP), np.float32)
        xk[:PC, :IN] = x[k * PC:(k + 1) * PC]
        xt = np.ascontiguousarray(xk.T).reshape(4, 128, PCP)
        in_maps.append(dict(
            xt=xt, w_in=w_in_p, b_in=b_in_p, att=att_p,
            w_cls=w_cls_p, b_cls=b_cls_p,
            gidx=per_core[k]["gidx"], dstloc=per_core[k]["dstloc"],
            dinv=per_core[k]["dinv"]))

    trace = _enable_trace()
    import tempfile
    res = run_bass_kernel_spmd(nc, in_maps, core_ids=list(range(NC_)),
                               trace=trace, tmpdir=tempfile.mkdtemp())
    LAST_EXEC_NS = res.exec_time_ns

    logits = np.zeros((N, C), np.float32)
    emb = np.zeros((N, H), np.float32)
    soft = np.zeros((N, C), np.float32)
    hard = np.zeros((N,), np.int32)
    for k in range(NC_):
        r = res.results[k]
        logits[k * PC:(k + 1) * PC] = r["logits"][:PC]
        emb[k * PC:(k + 1) * PC] = r["emb"][:PC]
        soft[k * PC:(k + 1) * PC] = r["soft"][:PC]
        hard[k * PC:(k + 1) * PC] = r["hard"][:PC, 0].astype(np.int32)
    return logits, emb, soft, hard


# revision 10
# speedup vs baseline: 1.1282x; 1.0568x over previous
"""FAGCN (2-layer FAConv GNN) Trainium2 kernel, 8 NeuronCores SPMD.

Sharding: nodes by id-range across 8 cores (12500 each); edges partitioned by
dst so segment-sum is local; per-layer halo exchange = AllGather of the
per-node table; small weights replicated.

Table row (fp16 x 256 = 512B): [dinv*h hi (128) | al hi | al lo | dinv*h lo
(126, features 0..125)] - hi+lo fp16 pairs carry ~22-bit mantissa (~f32).
dinv_src is folded into the table rows and dinv_dst into the PSUM evacuation,
so the per-edge coefficient is just tanh(al_src + ar_dst).

Per-core pipeline:
  A) h = x @ W_in + b_in (f32 PE matmuls from host-transposed x), h0e=eps*h
  B) per layer: al/ar row-dots, build table shard -> AllGather -> full table
  C) per layer, per (supertile x src-bank) call: dma_gather 512B rows by edge
     src (int16 bank-local ids), reconstruct f32 messages, Z = ar_row + al_s,
     tanh on ScalarE, M = onehot(dstloc) * tanh(Z), f32 one-hot matmul
     segment-sum into PSUM, evac dinv_d*agg + eps*h0 (+relu for layer 1)
  D) logits / softmax / argmax head
"""
import sys
import numpy as np

for _p in ('/opt/trn_rl_repo', '/root/.axon_site'):
    if _p not in sys.path:
        sys.path.insert(0, _p)

from concourse import bass, mybir  # noqa: E402
import concourse.tile as tile  # noqa: E402
from concourse import bacc  # noqa: E402
from concourse.masks import make_identity  # noqa: E402
from concourse.bass_utils import run_bass_kernel_spmd  # noqa: E402

F32 = mybir.dt.float32
F16 = mybir.dt.float16
I16 = mybir.dt.int16
U32 = mybir.dt.uint32
AF = mybir.ActivationFunctionType
ALU = mybir.AluOpType

N, E, IN, H, C = 100000, 1600000, 500, 128, 40
EPS = 0.1
NC_ = 8                     # cores
PC = 12500                  # real nodes per core
NT = 98                     # dst tiles per core
PCP = NT * 128              # 12544 padded nodes per core
GN = NC_ * PCP              # 100352 padded global rows
NBANK = 4
BROWS = GN // NBANK         # 25088 rows per bank (< 32768 for int16 idx)
SUP = 4                     # dst tiles per supertile
RW = 256                    # fp16 elems per table row (512B)
INP = 512                   # padded input dim

LAST_EXEC_NS = None


def _enable_trace():
    try:
        import types
        import antenv
        if 'antenv.axon_hooks' not in sys.modules:
            hm = types.ModuleType('antenv.axon_hooks')
            _h = {}
            hm.set_axon_ntff_profile_hook = lambda h: _h.__setitem__('h', h)
            hm.get_axon_ntff_profile_hook = lambda: _h.get('h')
            sys.modules['antenv.axon_hooks'] = hm
            antenv.axon_hooks = hm
            from trn_agent_boot.trn_boot import _ntff_profile_via_ctypes
            hook = _ntff_profile_via_ctypes('/opt/axon/libaxon_pjrt.so')
            if hook is not None:
                hm.set_axon_ntff_profile_hook(hook)
        return sys.modules['antenv.axon_hooks'].get_axon_ntff_profile_hook() is not None
    except Exception:
        return False


def _wrap_idxs(idx):
    n = len(idx)
    S = -(-n // 16)
    flat = np.zeros(S * 16, np.int64)
    flat[:n] = idx
    buf = flat.reshape(S, 16).T.astype(np.int16)
    return np.tile(buf, (8, 1))


def _prep_edges(edge_index):
    src = np.concatenate([edge_index[0], np.arange(N, dtype=np.int64)])
    dst = np.concatenate([edge_index[1], np.arange(N, dtype=np.int64)])
    deg = np.bincount(dst, minlength=N).astype(np.float64)
    dinv = (1.0 / np.sqrt(deg)).astype(np.float32)
    row = (src // PC) * PCP + (src % PC)
    core = dst // PC

    NS = (NT + SUP - 1) // SUP
    counts = np.zeros((NC_, NS, NBANK, SUP), np.int64)
    buckets = [[[[None] * SUP for _ in range(NBANK)] for _ in range(NS)]
               for _ in range(NC_)]
    for k in range(NC_):
        m = core == k
        er, ed = row[m], (dst[m] - k * PC).astype(np.int64)
        t = ed >> 7
        b = er // BROWS
        key = (t // SUP) * (NBANK * SUP) + b * SUP + (t % SUP)
        order = np.argsort(key, kind='stable')
        er, ed, key = er[order], ed[order], key[order]
        bnd = np.searchsorted(key, np.arange(NS * NBANK * SUP + 1))
        for s in range(NS):
            for b_ in range(NBANK):
                for ti in range(SUP):
                    kk = s * (NBANK * SUP) + b_ * SUP + ti
                    lo, hi = bnd[kk], bnd[kk + 1]
                    counts[k, s, b_, ti] = hi - lo
                    buckets[k][s][b_][ti] = (er[lo:hi] - b_ * BROWS,
                                             ed[lo:hi] - (s * SUP + ti) * 128)
    mx = counts.max(axis=0)
    ntile = -(-mx // 128)
    calls = []
    tile_off = 0
    idx_coloff = 0
    for s in range(NS):
        sup_w = min(SUP, NT - s * SUP)
        for b_ in range(NBANK):
            tl = [int(ntile[s, b_, ti]) for ti in range(sup_w)]
            ntl = sum(tl)
            if ntl == 0:
                continue
            calls.append((s, b_, tile_off, tl, idx_coloff, ntl * 128))
            tile_off += ntl
            idx_coloff += ntl * 8
    TT, GW = tile_off, idx_coloff
    per_core = []
    for k in range(NC_):
        gidx = np.zeros((128, GW), np.int16)
        dstloc = np.zeros((128, TT), np.float32)
        for (s, b_, toff, tl, ioff, nidx) in calls:
            ii = np.zeros(nidx, np.int64)
            dd = np.full(nidx, -1.0, np.float64)   # pad edges match no column
            pos = 0
            for ti, ntl_t in enumerate(tl):
                er, ed = buckets[k][s][b_][ti]
                ii[pos:pos + len(er)] = er
                dd[pos:pos + len(ed)] = ed
                pos += ntl_t * 128
            gidx[:, ioff:ioff + nidx // 16] = _wrap_idxs(ii)
            dstloc[:, toff:toff + nidx // 128] = dd.reshape(-1, 128).T
        per_core.append(dict(gidx=gidx, dstloc=dstloc,
                             dinv=np.ascontiguousarray(
                                 np.pad(dinv[k * PC:(k + 1) * PC],
                                        (0, PCP - PC), constant_values=1.0)
                                 .reshape(NT, 128).T)))
    return dict(calls=calls, TT=TT, GW=GW, NS=NS), per_core


def _build_nc(meta):
    nc = bacc.Bacc("TRN2", target_bir_lowering=False, debug=False,
                   num_devices=NC_, num_swdge_queues=4)
    xt = nc.dram_tensor("xt", [4, 128, PCP], F32, kind="ExternalInput")
    w_in = nc.dram_tensor("w_in", [4, 128, H], F32, kind="ExternalInput")
    b_in_d = nc.dram_tensor("b_in", [128, 1], F32, kind="ExternalInput")
    att_d = nc.dram_tensor("att", [4, 128, H], F32, kind="ExternalInput")
    w_cls_d = nc.dram_tensor("w_cls", [128, C], F32, kind="ExternalInput")
    b_cls_d = nc.dram_tensor("b_cls", [128, C], F32, kind="ExternalInput")
    gidx_d = nc.dram_tensor("gidx", [128, meta["GW"]], I16, kind="ExternalInput")
    dstloc_d = nc.dram_tensor("dstloc", [128, meta["TT"]], F32,
                              kind="ExternalInput")
    dinv_d = nc.dram_tensor("dinv", [128, NT], F32, kind="ExternalInput")
    logits_d = nc.dram_tensor("logits", [PCP, C], F32, kind="ExternalOutput")
    emb_d = nc.dram_tensor("emb", [PCP, H], F32, kind="ExternalOutput")
    soft_d = nc.dram_tensor("soft", [PCP, C], F32, kind="ExternalOutput")
    hard_d = nc.dram_tensor("hard", [PCP, 8], U32, kind="ExternalOutput")

    calls = meta["calls"]
    last_mm = {}
    for ci, (s, b_, toff, tl, ioff, nidx) in enumerate(calls):
        for ti, w in enumerate(tl):
            if w > 0:
                last_mm[(s, ti)] = (ci, ti, w - 1)

    with tile.TileContext(nc) as tc:
        with tc.tile_pool(name="persist", bufs=1) as pp, \
             tc.tile_pool(name="dram", bufs=1, space="DRAM") as dp:
            cur_nm = pp.tile([128, NT, H], F32, tag="cur")
            dstloc_sb = pp.tile([128, meta["TT"]], F32, tag="dstloc")
            dinv_sb = pp.tile([128, NT], F32, tag="dinv")
            iota_sb = pp.tile([128, 128], F32, tag="iota")
            ident = pp.tile([128, 128], F32, tag="ident")
            atts = [pp.tile([128, H], F32, tag=f"att{i}", name=f"att{i}")
                    for i in range(4)]
            b_in_sb = pp.tile([128, 1], F32, tag="b_in")
            w_cls_sb = pp.tile([128, C], F32, tag="w_cls")
            b_cls_sb = pp.tile([128, C], F32, tag="b_cls")
            ar_nm = pp.tile([128, NT], F32, tag="ar_nm")
            gidx_sb = pp.tile([128, meta["GW"]], I16, tag="gidx")
            nc.sync.dma_start(out=gidx_sb[:], in_=gidx_d[:])
            al_nm = pp.tile([128, NT], F32, tag="al_nm")
            al_hi_nm = pp.tile([128, NT], F16, tag="al_hi")
            al_lo_nm = pp.tile([128, NT], F16, tag="al_lo")

            nc.sync.dma_start(out=dstloc_sb[:], in_=dstloc_d[:])
            nc.sync.dma_start(out=dinv_sb[:], in_=dinv_d[:])
            for i in range(4):
                nc.sync.dma_start(out=atts[i][:], in_=att_d[i])
            nc.sync.dma_start(out=b_in_sb[:], in_=b_in_d[:])
            nc.sync.dma_start(out=w_cls_sb[:], in_=w_cls_d[:])
            nc.sync.dma_start(out=b_cls_sb[:], in_=b_cls_d[:])
            nc.gpsimd.iota(iota_sb[:], pattern=[[1, 128]], base=0,
                           channel_multiplier=0,
                           allow_small_or_imprecise_dtypes=True)
            make_identity(nc, ident[:])

            h0e_d = dp.tile([PCP, H], F32, tag="h0e")      # eps * h0
            arf_d = dp.tile([1, PCP], F32, tag="arf")      # ar row-major
            shard1 = dp.tile([PCP, RW], F16, tag="shard1")
            table1 = dp.tile([GN, RW], F16, tag="table1")
            shard2 = dp.tile([PCP, RW], F16, tag="shard2")
            table2 = dp.tile([GN, RW], F16, tag="table2")

            # ---------- Phase A ----------
            with tc.tile_pool(name="pha", bufs=3) as pa, \
                 tc.tile_pool(name="phaps", bufs=2, space="PSUM") as paps, \
                 tc.tile_pool(name="phaw", bufs=1) as paw:
                ws = [paw.tile([128, H], F32, tag=f"w{c}", name=f"w{c}")
                      for c in range(4)]
                for c in range(4):
                    nc.sync.dma_start(out=ws[c][:], in_=w_in[c])
                cols_list = [(j * 512, 512) for j in range(PCP // 512)]
                if PCP % 512:
                    cols_list.append((PCP - PCP % 512, PCP % 512))
                for (c0, cw) in cols_list:
                    ps = paps.tile([128, 512], F32, tag="hps")
                    xts = []
                    for c in range(4):
                        xc = pa.tile([128, 512], F32, tag=f"x{c}", name=f"xc{c}")
                        nc.sync.dma_start(out=xc[:, :cw], in_=xt[c, :, c0:c0 + cw])
                        xts.append(xc)
                    for c in range(4):
                        nc.tensor.matmul(out=ps[:, :cw], lhsT=ws[c][:],
                                         rhs=xts[c][:, :cw],
                                         start=(c == 0), stop=(c == 3))
                    hT = pa.tile([128, 512], F32, tag="hT")
                    nc.vector.tensor_scalar(out=hT[:, :cw], in0=ps[:, :cw],
                                            scalar1=b_in_sb[:], scalar2=None,
                                            op0=ALU.add)
                    for bblk in range(cw // 128):
                        t_glob = (c0 + bblk * 128) // 128
                        pt = paps.tile([128, 128], F32, tag="tps")
                        nc.tensor.transpose(
                            out=pt[:], in_=hT[:, bblk * 128:(bblk + 1) * 128],
                            identity=ident[:])
                        nc.vector.tensor_copy(out=cur_nm[:, t_glob, :], in_=pt[:])
                        h0e = pa.tile([128, H], F32, tag="h0e")
                        nc.vector.tensor_scalar(out=h0e[:],
                                                in0=cur_nm[:, t_glob, :],
                                                scalar1=EPS, scalar2=None,
                                                op0=ALU.mult)
                        nc.sync.dma_start(
                            out=h0e_d[t_glob * 128:(t_glob + 1) * 128, :],
                            in_=h0e[:])

            def rowdot(dst_tile_col, att_t, pool, t0, tw):
                tmp = pool.tile([128, 16, H], F32, tag="rd_tmp")
                nc.vector.tensor_tensor(
                    out=tmp[:, :tw, :], in0=cur_nm[:, t0:t0 + tw, :],
                    in1=att_t[:].rearrange("p (o f) -> p o f", o=1)
                    .to_broadcast([128, tw, H]),
                    op=ALU.mult)
                nc.vector.tensor_reduce(
                    out=dst_tile_col[:, t0:t0 + tw], in_=tmp[:, :tw, :],
                    axis=mybir.AxisListType.X, op=ALU.add)

            def build_layer(layer, shard_t, table_t, al_i, ar_i):
                with tc.tile_pool(name=f"rd{layer}", bufs=2) as rp:
                    for t0 in range(0, NT, 16):
                        tw = min(16, NT - t0)
                        rowdot(al_nm, atts[al_i], rp, t0, tw)
                        rowdot(ar_nm, atts[ar_i], rp, t0, tw)
                # ar -> row-major DRAM (for per-dst-tile broadcast loads)
                nc.sync.dma_start(
                    out=arf_d[:].rearrange("o (t p) -> p t o", p=128),
                    in_=ar_nm[:].rearrange("p (t o) -> p t o", o=1))
                # al hi/lo split (fp16 pair)
                nc.vector.tensor_copy(out=al_hi_nm[:], in_=al_nm[:])
                nc.vector.tensor_tensor(out=al_lo_nm[:], in0=al_nm[:],
                                        in1=al_hi_nm[:], op=ALU.subtract)
                # table shard: [dinv*h hi | al hi | al lo | dinv*h lo(0:126)]
                with tc.tile_pool(name=f"asm{layer}", bufs=3) as ap_:
                    for t in range(NT):
                        asm = ap_.tile([128, RW], F16, tag="asm")
                        nc.vector.tensor_scalar(
                            out=asm[:, 0:H], in0=cur_nm[:, t, :],
                            scalar1=dinv_sb[:, t:t + 1], scalar2=None,
                            op0=ALU.mult)
                        nc.vector.scalar_tensor_tensor(
                            out=asm[:, H + 2:RW], in0=cur_nm[:, t, 0:126],
                            scalar=dinv_sb[:, t:t + 1],
                            in1=asm[:, 0:126], op0=ALU.mult, op1=ALU.subtract)
                        nc.vector.tensor_copy(out=asm[:, H:H + 1],
                                              in_=al_hi_nm[:, t:t + 1])
                        nc.vector.tensor_copy(out=asm[:, H + 1:H + 2],
                                              in_=al_lo_nm[:, t:t + 1])
                        nc.sync.dma_start(out=shard_t[t * 128:(t + 1) * 128, :],
                                          in_=asm[:])
                nc.gpsimd.collective_compute(
                    "AllGather", ALU.bypass,
                    replica_groups=[list(range(NC_))],
                    ins=[shard_t.opt()], outs=[table_t.opt()])

                with tc.tile_pool(name=f"ed{layer}", bufs=4) as ep, \
                     tc.tile_pool(name=f"edm{layer}", bufs=4) as mp, \
                     tc.tile_pool(name=f"edps{layer}", bufs=2,
                                  space="PSUM") as pps, \
                     tc.tile_pool(name=f"ev{layer}", bufs=3) as vp, \
                     tc.tile_pool(name=f"arr{layer}", bufs=2) as arp:

                    def evac(s, psums):
                        sup_w = min(SUP, NT - s * SUP)
                        for ti in range(sup_w):
                            t_glob = s * SUP + ti
                            ps = psums[ti]
                            h0t = vp.tile([128, H], F32, tag="h0t")
                            nc.sync.dma_start(
                                out=h0t[:],
                                in_=h0e_d[t_glob * 128:(t_glob + 1) * 128, :])
                            if layer == 1:
                                tmp = vp.tile([128, H], F32, tag="ev")
                                nc.vector.scalar_tensor_tensor(
                                    out=tmp[:], in0=ps[:],
                                    scalar=dinv_sb[:, t_glob:t_glob + 1],
                                    in1=h0t[:], op0=ALU.mult, op1=ALU.add)
                                nc.scalar.activation(out=cur_nm[:, t_glob, :],
                                                     in_=tmp[:], func=AF.Relu)
                            else:
                                nc.vector.scalar_tensor_tensor(
                                    out=cur_nm[:, t_glob, :], in0=ps[:],
                                    scalar=dinv_sb[:, t_glob:t_glob + 1],
                                    in1=h0t[:], op0=ALU.mult, op1=ALU.add)

                    s_cur = -1
                    psums = {}
                    started = {}
                    arreps = {}
                    qn = 0
                    for ci, (s, b_, toff, tl, ioff, nidx) in enumerate(calls):
                        if s != s_cur:
                            if s_cur >= 0:
                                evac(s_cur, psums)
                            psums = {}
                            started = {}
                            arreps = {}
                            s_cur = s
                            for ti in range(len(tl)):
                                psums[ti] = pps.tile(
                                    [128, H], F32, tag=f"ps{ti}",
                                    name=f"ps_{layer}_{s}_{ti}")
                                t_glob = s * SUP + ti
                                arr = arp.tile([128, 128], F32, tag=f"ar{ti}",
                                               name=f"ar_{layer}_{s}_{ti}")
                                nc.sync.dma_start(
                                    out=arr[:],
                                    in_=arf_d[0:1,
                                              t_glob * 128:(t_glob + 1) * 128]
                                    .to_broadcast([128, 128]))
                                arreps[ti] = arr
                        ntl = sum(tl)
                        tg = ep.tile([128, ntl, RW], F16, tag="gat")
                        nc.gpsimd.dma_gather(
                            tg[:], table_t[b_ * BROWS:(b_ + 1) * BROWS, :],
                            gidx_sb[:, ioff:ioff + nidx // 16],
                            nidx, nidx, RW,
                            single_packet=False, queue_num=qn)
                        qn = (qn + 1) % 4
                        # f32 messages: hi + lo (features 0..125), hi (126..7)
                        msg = mp.tile([128, ntl, H], F32, tag="msg")
                        nc.vector.tensor_tensor(
                            out=msg[:, :, 0:126], in0=tg[:, :, 0:126],
                            in1=tg[:, :, H + 2:RW], op=ALU.add)
                        nc.vector.tensor_copy(out=msg[:, :, 126:128],
                                              in_=tg[:, :, 126:128])
                        # al_s (f32) per edge
                        alv = mp.tile([128, ntl], F32, tag="alv")
                        nc.vector.tensor_tensor(
                            out=alv[:].rearrange("p (n o) -> p n o", o=1),
                            in0=tg[:, :, H:H + 1], in1=tg[:, :, H + 1:H + 2],
                            op=ALU.add)
                        # per tile: tanh(ar_row + al_s) on ScalarE (bias
                        # = per-edge al), then M = onehot(dstloc)*tanh(Z) in
                        # one fused DVE op, then the segment-sum matmul
                        r0 = 0
                        for ti, w in enumerate(tl):
                            for j in range(w):
                                zt = mp.tile([128, 128], F32, tag="zt",
                                             name=f"zt_{layer}_{ci}_{r0+j}")
                                nc.scalar.activation(
                                    out=zt[:], in_=arreps[ti][:],
                                    func=AF.Tanh,
                                    bias=alv[:, r0 + j:r0 + j + 1])
                                mb = mp.tile([128, 128], F32, tag="mb",
                                             name=f"mb_{layer}_{ci}_{r0+j}")
                                nc.vector.scalar_tensor_tensor(
                                    out=mb[:], in0=iota_sb[:],
                                    scalar=dstloc_sb[:,
                                                     toff + r0 + j:
                                                     toff + r0 + j + 1],
                                    in1=zt[:], op0=ALU.is_equal, op1=ALU.mult)
                                first = not started.get(ti, False)
                                started[ti] = True
                                stop = last_mm.get((s, ti)) == (ci, ti, j)
                                nc.tensor.matmul(
                                    out=psums[ti][:],
                                    lhsT=mb[:],
                                    rhs=msg[:, r0 + j, :],
                                    start=first, stop=stop)
                            r0 += w
                    if s_cur >= 0:
                        evac(s_cur, psums)

            build_layer(1, shard1, table1, 0, 1)
            build_layer(2, shard2, table2, 2, 3)

            # ---------- Phase D: head ----------
            with tc.tile_pool(name="hd", bufs=2) as hp, \
                 tc.tile_pool(name="hdps", bufs=4, space="PSUM") as hps:
                for t0 in range(0, NT, 14):
                    tw = min(14, NT - t0)
                    lg = hp.tile([128, 14, C], F32, tag="lg")
                    for ti in range(tw):
                        t = t0 + ti
                        tp_ = hps.tile([128, H], F32, tag="tp")
                        nc.tensor.transpose(out=tp_[:], in_=cur_nm[:, t, :],
                                            identity=ident[:])
                        h2T = hp.tile([128, H], F32, tag="h2T")
                        nc.vector.tensor_copy(out=h2T[:], in_=tp_[:])
                        lp = hps.tile([128, C], F32, tag="lp")
                        nc.tensor.matmul(out=lp[:], lhsT=h2T[:],
                                         rhs=w_cls_sb[:], start=True, stop=True)
                        nc.vector.tensor_tensor(out=lg[:, ti, :], in0=lp[:],
                                                in1=b_cls_sb[:], op=ALU.add)
                        nc.sync.dma_start(out=emb_d[t * 128:(t + 1) * 128, :],
                                          in_=cur_nm[:, t, :])
                    nc.sync.dma_start(
                        out=logits_d.ap().rearrange("(t p) c -> p t c", p=128)
                        [:, t0:t0 + tw, :],
                        in_=lg[:, :tw, :])
                    mx = hp.tile([128, 14], F32, tag="mx")
                    nc.vector.tensor_reduce(out=mx[:, :tw], in_=lg[:, :tw, :],
                                            axis=mybir.AxisListType.X,
                                            op=ALU.max)
                    ex = hp.tile([128, 14, C], F32, tag="ex")
                    nc.vector.tensor_tensor(
                        out=ex[:, :tw, :], in0=lg[:, :tw, :],
                        in1=mx[:, :tw].to_broadcast([128, tw, C]),
                        op=ALU.subtract)
                    nc.scalar.activation(out=ex[:, :tw, :], in_=ex[:, :tw, :],
                                         func=AF.Exp)
                    sm = hp.tile([128, 14], F32, tag="sm")
                    nc.vector.tensor_reduce(out=sm[:, :tw], in_=ex[:, :tw, :],
                                            axis=mybir.AxisListType.X,
                                            op=ALU.add)
                    rc = hp.tile([128, 14], F32, tag="rc")
                    nc.vector.reciprocal(out=rc[:, :tw], in_=sm[:, :tw])
                    nc.vector.tensor_tensor(
                        out=ex[:, :tw, :], in0=ex[:, :tw, :],
                        in1=rc[:, :tw].to_broadcast([128, tw, C]),
                        op=ALU.mult)
                    nc.sync.dma_start(
                        out=soft_d.ap().rearrange("(t p) c -> p t c", p=128)
                        [:, t0:t0 + tw, :],
                        in_=ex[:, :tw, :])
                    hmx = hp.tile([128, 14, 8], F32, tag="hmx")
                    hix = hp.tile([128, 14, 8], U32, tag="hix")
                    for ti in range(tw):
                        nc.vector.max(out=hmx[:, ti, :], in_=lg[:, ti, :])
                        nc.vector.max_index(out=hix[:, ti, :],
                                            in_max=hmx[:, ti, :],
                                            in_values=lg[:, ti, :])
                    nc.sync.dma_start(
                        out=hard_d.ap().rearrange("(t p) c -> p t c", p=128)
                        [:, t0:t0 + tw, :],
                        in_=hix[:, :tw, :])
    nc.compile()
    return nc


def kernel(x, edge_index, W_in, b_in, att_l1, att_r1, att_l2, att_r2,
           W_cls, b_cls):
    global LAST_EXEC_NS
    x = np.asarray(x)
    edge_index = np.asarray(edge_index)
    meta, per_core = _prep_edges(edge_index)
    nc = _build_nc(meta)

    w_in_p = np.zeros((4, 128, H), np.float32)
    w_in_p.reshape(512, H)[:IN] = np.asarray(W_in, np.float32)
    b_in_p = np.asarray(b_in, np.float32).reshape(128, 1)
    att_p = np.stack([np.tile(np.asarray(a, np.float32)[None, :], (128, 1))
                      for a in (att_l1, att_r1, att_l2, att_r2)])
    w_cls_p = np.asarray(W_cls, np.float32)
    b_cls_p = np.tile(np.asarray(b_cls, np.float32)[None, :], (128, 1))

    in_maps = []
    for k in range(NC_):
        xk = np.zeros((PCP, INP), np.float32)
        xk[:PC, :IN] = x[k * PC:(k + 1) * PC]
        xt = np.ascontiguousarray(xk.T).reshape(4, 128, PCP)
        in_maps.append(dict(
            xt=xt, w_in=w_in_p, b_in=b_in_p, att=att_p,
            w_cls=w_cls_p, b_cls=b_cls_p,
            gidx=per_core[k]["gidx"], dstloc=per_core[k]["dstloc"],
            dinv=per_core[k]["dinv"]))

    trace = _enable_trace()
    import tempfile
    res = run_bass_kernel_spmd(nc, in_maps, core_ids=list(range(NC_)),
                               trace=trace, tmpdir=tempfile.mkdtemp())
    LAST_EXEC_NS = res.exec_time_ns

    logits = np.zeros((N, C), np.float32)
    emb = np.zeros((N, H), np.float32)
    soft = np.zeros((N, C), np.float32)
    hard = np.zeros((N,), np.int32)
    for k in range(NC_):
        r = res.results[k]
        logits[k * PC:(k + 1) * PC] = r["logits"][:PC]
        emb[k * PC:(k + 1) * PC] = r["emb"][:PC]
        soft[k * PC:(k + 1) * PC] = r["soft"][:PC]
        hard[k * PC:(k + 1) * PC] = r["hard"][:PC, 0].astype(np.int32)
    return logits, emb, soft, hard
